# revision 11
# baseline (speedup 1.0000x reference)
"""GATv2Conv forward on 8 TRN2 NeuronCores (Bass/Tile).

Math restructuring (equivalent to the reference up to fp rounding):
  logits_e = lrelu(f_e) @ a1 + lrelu(x[dst_e]) @ a2,  f_e = x[src_e] @ W1x^T + ea_e @ W1e^T
  The a2 term is constant within each dst softmax segment -> cancels in softmax.
  alpha_e = softmax_seg(l_e) with l_e = lrelu(f_e) @ a1 (shift-invariant; |l| is
  O(1) here so exp() needs no max pass).
  h[n] = (sum_e p_e * x_t[src_e]) / (sum_e p_e) + bias,  p_e = exp(l_e)
  (the softmax division commutes with the segment sum -> one pass over edges).
  With the SIGNED a1 folded into W1 on the host (f'' = f * a1, features permuted
  so a1>=0 comes first):
    l = sum_{k<P} max(f''_k, 0.01 f''_k) + sum_{k>=P} min(f''_k, 0.01 f''_k)
  each half is one DVE scalar_tensor_tensor((f''*0.01) max/min f'') with accum_out.

Distribution: edges sharded by dst range (core c owns nodes [c*12500,(c+1)*12500)),
so each core computes final output rows for its range with no collectives.
x and weights are replicated. The host orders each shard's edges by 256-node dst
window; aggregation is a one-hot matmul into PSUM per window (selection matrix
via is_equal against an iota row, bf16). Per-edge rows [u''(64) | 1 | x_t(64)]
are fetched from a device-built DRAM table by indirect DMA (128 edges per
instruction — the machine's max), and the whole payload [p | p*x_t] is produced
by one scale-by-p copy of columns 64:129.

Host-side work is only sharding/scheduling: bucketing edges by dst window,
padding windows to 128-edge tiles (dummy edges point at a sentinel table row
whose u'' = -1000 everywhere -> logit <= -640 -> p = 0 -> zero payload), and
equalizing the per-window tile counts across cores so one SPMD graph serves
all 8 cores.
"""
import os
import sys

for _p in ("/opt/trn_rl_repo", "/root/.axon_site/_ro/trn_rl_repo"):
    if os.path.isdir(_p) and _p not in sys.path:
        sys.path.insert(0, _p)

import numpy as np

# ---- problem constants (hardcoded per the task spec) ----
N = 100000
E = 1280000
D_IN = 64
D_OUT = 64
D_EDGE = 16
NEG = 0.01
N_CORES = 8
NPC = N // N_CORES          # nodes per core
WIN = 256                   # dst window (one PSUM aggregation block)
NWIN = (NPC + WIN - 1) // WIN
TROWS = ((N + 1 + 511) // 512) * 512   # gather-table rows (incl sentinel), padded
SENT = N                    # sentinel table row index
TW = 132                    # table row stride (floats): u''(64) | 1 | xt(64) | pad(3)
GW = 2 * D_OUT + 1          # gathered row width (129)
EA_CH = 32                  # ea chunk size in tiles
PAN_CH = 256                # index/dstw panel chunk size in tiles


def _preprocess(x, edge_attr, src, dst, W1, W2, attn, bias):
    """Shard + schedule on host. Returns (in_maps, meta)."""
    import ml_dtypes
    bf16 = ml_dtypes.bfloat16
    a1 = np.asarray(attn[:D_OUT], np.float64)
    perm = np.argsort(a1 < 0, kind="stable")  # nonneg signs first
    P = int((a1 >= 0).sum())
    a1p = a1[perm].astype(np.float32)         # signed fold
    W1x = W1[:, :D_IN]
    W1e = W1[:, D_IN:]
    W1xp = (W1x[perm, :] * a1p[:, None]).astype(np.float32)   # [64,64]
    W1ep = (W1e[perm, :] * a1p[:, None]).astype(np.float32)   # [64,16]
    Wcomb = np.concatenate([W1xp.T, W2.T.astype(np.float32)], axis=1)  # [64,128]
    W1eT = np.ascontiguousarray(W1ep.T)                        # [16,64]

    xT = np.zeros((D_IN, TROWS), np.float32)
    xT[:, :N] = np.asarray(x, np.float32).T
    sent_row = np.zeros((1, TW), np.float32)
    sent_row[0, :D_OUT] = -1000.0   # logit <= -640 -> exp == 0 (xt cols stay 0)

    iota_row = np.tile(np.arange(WIN, dtype=bf16), (128, 1))
    bias_rep = np.tile(np.asarray(bias, np.float32)[None, :], (128, 1))

    src = np.asarray(src)
    dst = np.asarray(dst)
    core_of = dst // NPC
    per_core = []
    counts = np.zeros((N_CORES, NWIN), np.int64)
    for c in range(N_CORES):
        sel = np.nonzero(core_of == c)[0]
        dl = dst[sel] - c * NPC
        w = dl // WIN
        order = np.argsort(w, kind="stable")
        sel = sel[order]
        cnt = np.bincount(dl[order] // WIN, minlength=NWIN)
        counts[c] = cnt
        per_core.append((sel, cnt))
    twin = np.maximum(1, (counts.max(axis=0) + 127) // 128).astype(np.int64)
    woff = np.concatenate([[0], np.cumsum(twin)])
    T = int(woff[-1])

    ea = np.asarray(edge_attr, np.float32)
    in_maps = []
    for c in range(N_CORES):
        sel, cnt = per_core[c]
        srcpan = np.full(T * 128, SENT, np.int32)
        dstwpan = np.zeros(T * 128, bf16)
        eaT = np.zeros((D_EDGE, T * 128), bf16)
        pos = 0
        for w in range(NWIN):
            s0 = int(woff[w]) * 128
            k = int(cnt[w])
            es = sel[pos:pos + k]
            pos += k
            srcpan[s0:s0 + k] = src[es]
            dstwpan[s0:s0 + k] = (dst[es] - c * NPC - w * WIN).astype(bf16)
            eaT[:, s0:s0 + k] = ea[es].T.astype(bf16)
        in_maps.append({
            "xT": xT.astype(bf16),
            "Wcomb": Wcomb.astype(bf16),
            "W1eT": W1eT.astype(bf16),
            "sent_row": sent_row,
            "iota_row": iota_row,
            "bias_rep": bias_rep,
            "eaT": eaT,
            "srcpan": np.ascontiguousarray(srcpan.reshape(T, 128).T),
            "dstwpan": np.ascontiguousarray(dstwpan.reshape(T, 128).T),
        })
    meta = {"T": T, "twin": twin.tolist(), "woff": woff.tolist(), "P": P}
    return in_maps, meta


def _build(meta):
    import concourse.bass as bass
    import concourse.bacc as bacc
    import concourse.mybir as mybir
    import concourse.tile as tile
    from concourse.masks import make_identity

    T = meta["T"]
    twin = meta["twin"]
    woff = meta["woff"]
    P = meta["P"]
    fdt = mybir.dt.float32
    bdt = mybir.dt.bfloat16
    AF = mybir.ActivationFunctionType
    OP = mybir.AluOpType

    nc = bacc.Bacc("TRN2", target_bir_lowering=False, debug=False)
    xT_d = nc.dram_tensor("xT", [D_IN, TROWS], bdt, kind="ExternalInput")
    Wcomb_d = nc.dram_tensor("Wcomb", [D_IN, 2 * D_OUT], bdt, kind="ExternalInput")
    W1eT_d = nc.dram_tensor("W1eT", [D_EDGE, D_OUT], bdt, kind="ExternalInput")
    sent_d = nc.dram_tensor("sent_row", [1, TW], fdt, kind="ExternalInput")
    iota_d = nc.dram_tensor("iota_row", [128, WIN], bdt, kind="ExternalInput")
    bias_d = nc.dram_tensor("bias_rep", [128, D_OUT], fdt, kind="ExternalInput")
    eaT_d = nc.dram_tensor("eaT", [D_EDGE, T * 128], bdt, kind="ExternalInput")
    srcp_d = nc.dram_tensor("srcpan", [128, T], mybir.dt.int32, kind="ExternalInput")
    dstw_d = nc.dram_tensor("dstwpan", [128, T], bdt, kind="ExternalInput")
    table_d = nc.dram_tensor("table", [TROWS, TW], fdt)
    out_d = nc.dram_tensor("out", [NPC, D_OUT], fdt, kind="ExternalOutput")

    ntile_tab = TROWS // 128

    with tile.TileContext(nc) as tc:
        with tc.tile_pool(name="const", bufs=1) as cpool:
            Wcomb_s = cpool.tile([D_IN, 2 * D_OUT], bdt)
            nc.sync.dma_start(out=Wcomb_s[:], in_=Wcomb_d[:])
            W1eT_s = cpool.tile([D_EDGE, D_OUT], bdt)
            nc.sync.dma_start(out=W1eT_s[:], in_=W1eT_d[:])
            iota_s = cpool.tile([128, WIN], bdt)
            nc.sync.dma_start(out=iota_s[:], in_=iota_d[:])
            bias_s = cpool.tile([128, D_OUT], fdt)
            nc.sync.dma_start(out=bias_s[:], in_=bias_d[:])
            sent_s = cpool.tile([1, TW], fdt)
            nc.sync.dma_start(out=sent_s[:], in_=sent_d[:])
            ident = cpool.tile([128, 128], fdt)
            make_identity(nc, ident[:])

            # ---- phase 0: table[r] = [x@W1x''^T | 1 | x@W2^T | pad] ----
            with (
                tc.tile_pool(name="p0s", bufs=3) as p0s,
                tc.tile_pool(name="p0p", bufs=2, space="PSUM") as p0p,
            ):
                XB = 4  # table tiles per DMA group (keeps DMA count low)
                for g in range(ntile_tab // XB):
                    r0 = g * XB * 128
                    xt_t = p0s.tile([D_IN, XB * 128], bdt, tag="xt")
                    nc.sync.dma_start(out=xt_t[:], in_=xT_d[:, r0:r0 + XB * 128])
                    ps = p0p.tile([128, XB * 2 * D_OUT], fdt, tag="ps")
                    for k in range(XB):
                        nc.tensor.matmul(
                            out=ps[:, k * 2 * D_OUT:(k + 1) * 2 * D_OUT],
                            lhsT=xt_t[:, k * 128:(k + 1) * 128],
                            rhs=Wcomb_s[:], start=True, stop=True)
                    ob = p0s.tile([128, XB * TW], fdt, tag="ob")
                    obv = ob[:].rearrange("p (k w) -> p k w", k=XB)
                    psv = ps[:].rearrange("p (k d) -> p k d", k=XB)
                    nc.scalar.copy(out=obv[:, :, 0:D_OUT], in_=psv[:, :, 0:D_OUT])
                    nc.vector.memset(obv[:, :, D_OUT:D_OUT + 1], 1.0)
                    nc.vector.memset(obv[:, :, GW:TW], 0.0)
                    nc.scalar.copy(out=obv[:, :, D_OUT + 1:GW],
                                   in_=psv[:, :, D_OUT:2 * D_OUT])
                    nc.sync.dma_start(
                        out=table_d[r0:r0 + XB * 128, :].rearrange(
                            "(k p) w -> p k w", p=128),
                        in_=obv)
                nc.sync.dma_start(out=table_d[SENT:SENT + 1, :], in_=sent_s[:])

            # ---- edge phase ----
            with (
                tc.tile_pool(name="gat", bufs=8) as gat,
                tc.tile_pool(name="eap", bufs=2) as eap,
                tc.tile_pool(name="pan", bufs=2) as pan,
                tc.tile_pool(name="wrk", bufs=4) as wrk,
                tc.tile_pool(name="psf", bufs=2, space="PSUM") as psf,
                tc.tile_pool(name="psh", bufs=2, space="PSUM") as psh,
                tc.tile_pool(name="pst", bufs=1, space="PSUM") as pst,
                tc.tile_pool(name="drn", bufs=2) as drn,
            ):
                ea_ch = None
                src_ch = None
                dstw_ch = None
                for w in range(NWIN):
                    tw = twin[w]
                    t0 = woff[w]
                    psH = psh.tile([D_OUT + 1, WIN], fdt, tag="psH")
                    for j in range(tw):
                        t = t0 + j
                        if t % EA_CH == 0:
                            ea_ch = eap.tile([D_EDGE, EA_CH * 128], bdt, tag="ea")
                            hi = min((t + EA_CH) * 128, T * 128)
                            nc.sync.dma_start(out=ea_ch[:, :hi - t * 128],
                                              in_=eaT_d[:, t * 128:hi])
                        if t % PAN_CH == 0:
                            hi = min(t + PAN_CH, T)
                            src_ch = pan.tile([128, PAN_CH], mybir.dt.int32, tag="srcp")
                            nc.sync.dma_start(out=src_ch[:, :hi - t], in_=srcp_d[:, t:hi])
                            dstw_ch = pan.tile([128, PAN_CH], bdt, tag="dstwp")
                            nc.sync.dma_start(out=dstw_ch[:, :hi - t], in_=dstw_d[:, t:hi])
                        co = t % EA_CH
                        po = t % PAN_CH

                        uxt = gat.tile([128, GW], fdt, tag="uxt")
                        nc.gpsimd.indirect_dma_start(
                            out=uxt[:],
                            out_offset=None,
                            in_=table_d[:],
                            in_offset=bass.IndirectOffsetOnAxis(
                                ap=src_ch[:, po:po + 1], axis=0),
                        )
                        psF = psf.tile([128, D_OUT], fdt, tag="psF")
                        nc.tensor.matmul(out=psF[:],
                                         lhsT=ea_ch[:, co * 128:(co + 1) * 128],
                                         rhs=W1eT_s[:], start=True, stop=True)
                        fpr = wrk.tile([128, D_OUT], fdt, tag="fpr")
                        nc.vector.tensor_add(out=fpr[:], in0=psF[:], in1=uxt[:, :D_OUT])
                        st = wrk.tile([128, 4], fdt, tag="st")
                        scr = wrk.tile([128, D_OUT], fdt, tag="scr")
                        # l = sum max(f'',.01f'')[:P] + sum min(f'',.01f'')[P:]
                        if P > 0:
                            nc.vector.scalar_tensor_tensor(
                                out=scr[:, :P], in0=fpr[:, :P], scalar=NEG,
                                in1=fpr[:, :P], op0=OP.mult, op1=OP.max,
                                accum_out=st[:, 0:1])
                        else:
                            nc.vector.memset(st[:, 0:1], 0.0)
                        if P < D_OUT:
                            nc.vector.scalar_tensor_tensor(
                                out=scr[:, P:D_OUT], in0=fpr[:, P:D_OUT], scalar=NEG,
                                in1=fpr[:, P:D_OUT], op0=OP.mult, op1=OP.min,
                                accum_out=st[:, 1:2])
                        else:
                            nc.vector.memset(st[:, 1:2], 0.0)
                        nc.scalar.activation(out=st[:, 2:3], in_=st[:, 1:2],
                                             func=AF.Exp, bias=st[:, 0:1])
                        pay = wrk.tile([128, D_OUT + 1], bdt, tag="pay")
                        nc.scalar.activation(out=pay[:], in_=uxt[:, D_OUT:GW],
                                             func=AF.Copy, scale=st[:, 2:3])
                        oh = wrk.tile([128, WIN], bdt, tag="oh")
                        nc.vector.tensor_tensor(
                            out=oh[:],
                            in0=dstw_ch[:, po:po + 1].to_broadcast([128, WIN]),
                            in1=iota_s[:],
                            op=OP.is_equal)
                        nc.tensor.matmul(out=psH[:], lhsT=pay[:], rhs=oh[:],
                                         start=(j == 0), stop=(j == tw - 1))
                    # ---- drain window w (two 128-node halves) ----
                    sbH = drn.tile([D_OUT + 1, WIN], fdt, tag="sbH")
                    nc.scalar.copy(out=sbH[:], in_=psH[:])
                    for hh in range(WIN // 128):
                        r0 = w * WIN + hh * 128
                        if r0 >= NPC:
                            break
                        rows = min(128, NPC - r0)
                        psHT = pst.tile([128, D_OUT + 1], fdt, tag="psHT")
                        nc.tensor.transpose(
                            out=psHT[:], in_=sbH[:, hh * 128:(hh + 1) * 128],
                            identity=ident[:D_OUT + 1, :D_OUT + 1])
                        dn = drn.tile([128, 2], fdt, tag="dn")
                        nc.vector.tensor_scalar_max(out=dn[:, 0:1], in0=psHT[:, 0:1],
                                                    scalar1=1e-30)
                        nc.vector.reciprocal(out=dn[:, 1:2], in_=dn[:, 0:1])
                        ot = drn.tile([128, D_OUT], fdt, tag="ot")
                        nc.scalar.activation(out=ot[:], in_=psHT[:, 1:D_OUT + 1],
                                             func=AF.Copy, scale=dn[:, 1:2])
                        ot2 = drn.tile([128, D_OUT], fdt, tag="ot2")
                        nc.vector.tensor_add(out=ot2[:], in0=ot[:], in1=bias_s[:])
                        nc.sync.dma_start(out=out_d[r0:r0 + rows, :], in_=ot2[:rows, :])
    nc.compile()
    return nc


def kernel(**inputs):
    in_maps, meta = _preprocess(**inputs)
    nc = _build(meta)
    from concourse import bass_utils
    res = bass_utils.run_bass_kernel_spmd(nc, in_maps, core_ids=list(range(N_CORES)))
    h = np.concatenate([res.results[c]["out"] for c in range(N_CORES)], axis=0)
    return h.astype(np.float32)


# revision 12
# speedup vs baseline: 1.0026x; 1.0026x over previous
"""GATv2Conv forward on 8 TRN2 NeuronCores (Bass/Tile).

Math restructuring (equivalent to the reference up to fp rounding):
  logits_e = lrelu(f_e) @ a1 + lrelu(x[dst_e]) @ a2,  f_e = x[src_e] @ W1x^T + ea_e @ W1e^T
  The a2 term is constant within each dst softmax segment -> cancels in softmax.
  alpha_e = softmax_seg(l_e) with l_e = lrelu(f_e) @ a1 (shift-invariant; |l| is
  O(1) here so exp() needs no max pass).
  h[n] = (sum_e p_e * x_t[src_e]) / (sum_e p_e) + bias,  p_e = exp(l_e)
  (the softmax division commutes with the segment sum -> one pass over edges).
  With the SIGNED a1 folded into W1 on the host (f'' = f * a1, features permuted
  so a1>=0 comes first):
    l = sum_{k<P} max(f''_k, 0.01 f''_k) + sum_{k>=P} min(f''_k, 0.01 f''_k)
  each half is one DVE scalar_tensor_tensor((f''*0.01) max/min f'') with accum_out.

Distribution: edges sharded by dst range (core c owns nodes [c*12500,(c+1)*12500)),
so each core computes final output rows for its range with no collectives.
x and weights are replicated. The host orders each shard's edges by 256-node dst
window; aggregation is a one-hot matmul into PSUM per window (selection matrix
via is_equal against an iota row, bf16). Per-edge rows [u''(64) | 1 | x_t(64)]
are fetched from a device-built DRAM table by indirect DMA (128 edges per
instruction — the machine's max), and the whole payload [p | p*x_t] is produced
by one scale-by-p copy of columns 64:129.

Host-side work is only sharding/scheduling: bucketing edges by dst window,
padding windows to 128-edge tiles (dummy edges point at a sentinel table row
whose u'' = -1000 everywhere -> logit <= -640 -> p = 0 -> zero payload), and
equalizing the per-window tile counts across cores so one SPMD graph serves
all 8 cores.
"""
import os
import sys

for _p in ("/opt/trn_rl_repo", "/root/.axon_site/_ro/trn_rl_repo"):
    if os.path.isdir(_p) and _p not in sys.path:
        sys.path.insert(0, _p)

import numpy as np

# ---- problem constants (hardcoded per the task spec) ----
N = 100000
E = 1280000
D_IN = 64
D_OUT = 64
D_EDGE = 16
NEG = 0.01
N_CORES = 8
NPC = N // N_CORES          # nodes per core
WIN = 256                   # dst window (one PSUM aggregation block)
NWIN = (NPC + WIN - 1) // WIN
TROWS = ((N + 1 + 511) // 512) * 512   # gather-table rows (incl sentinel), padded
SENT = N                    # sentinel table row index
TW = 132                    # table row stride (floats): u''(64) | 1 | xt(64) | pad(3)
GW = 2 * D_OUT + 1          # gathered row width (129)
EA_CH = 32                  # ea chunk size in tiles
PAN_CH = 256                # index/dstw panel chunk size in tiles


def _preprocess(x, edge_attr, src, dst, W1, W2, attn, bias):
    """Shard + schedule on host. Returns (in_maps, meta)."""
    import ml_dtypes
    bf16 = ml_dtypes.bfloat16
    a1 = np.asarray(attn[:D_OUT], np.float64)
    perm = np.argsort(a1 < 0, kind="stable")  # nonneg signs first
    P = int((a1 >= 0).sum())
    a1p = a1[perm].astype(np.float32)         # signed fold
    W1x = W1[:, :D_IN]
    W1e = W1[:, D_IN:]
    W1xp = (W1x[perm, :] * a1p[:, None]).astype(np.float32)   # [64,64]
    W1ep = (W1e[perm, :] * a1p[:, None]).astype(np.float32)   # [64,16]
    Wcomb = np.concatenate([W1xp.T, W2.T.astype(np.float32)], axis=1)  # [64,128]
    W1eT = np.ascontiguousarray(W1ep.T)                        # [16,64]

    xT = np.zeros((D_IN, TROWS), np.float32)
    xT[:, :N] = np.asarray(x, np.float32).T
    sent_row = np.zeros((1, TW), bf16)
    sent_row[0, :D_OUT] = -1000.0   # logit <= -640 -> exp == 0 (xt cols stay 0)

    iota_row = np.tile(np.arange(WIN, dtype=bf16), (128, 1))
    bias_rep = np.tile(np.asarray(bias, np.float32)[None, :], (128, 1))

    src = np.asarray(src)
    dst = np.asarray(dst)
    core_of = dst // NPC
    per_core = []
    counts = np.zeros((N_CORES, NWIN), np.int64)
    for c in range(N_CORES):
        sel = np.nonzero(core_of == c)[0]
        dl = dst[sel] - c * NPC
        w = dl // WIN
        order = np.argsort(w, kind="stable")
        sel = sel[order]
        cnt = np.bincount(dl[order] // WIN, minlength=NWIN)
        counts[c] = cnt
        per_core.append((sel, cnt))
    twin = np.maximum(1, (counts.max(axis=0) + 127) // 128).astype(np.int64)
    woff = np.concatenate([[0], np.cumsum(twin)])
    T = int(woff[-1])

    ea = np.asarray(edge_attr, np.float32)
    in_maps = []
    for c in range(N_CORES):
        sel, cnt = per_core[c]
        srcpan = np.full(T * 128, SENT, np.int32)
        dstwpan = np.zeros(T * 128, bf16)
        eaT = np.zeros((D_EDGE, T * 128), bf16)
        pos = 0
        for w in range(NWIN):
            s0 = int(woff[w]) * 128
            k = int(cnt[w])
            es = sel[pos:pos + k]
            pos += k
            srcpan[s0:s0 + k] = src[es]
            dstwpan[s0:s0 + k] = (dst[es] - c * NPC - w * WIN).astype(bf16)
            eaT[:, s0:s0 + k] = ea[es].T.astype(bf16)
        in_maps.append({
            "xT": xT.astype(bf16),
            "Wcomb": Wcomb.astype(bf16),
            "W1eT": W1eT.astype(bf16),
            "sent_row": sent_row,
            "iota_row": iota_row,
            "bias_rep": bias_rep,
            "eaT": eaT,
            "srcpan": np.ascontiguousarray(srcpan.reshape(T, 128).T),
            "dstwpan": np.ascontiguousarray(dstwpan.reshape(T, 128).T),
        })
    meta = {"T": T, "twin": twin.tolist(), "woff": woff.tolist(), "P": P}
    return in_maps, meta


def _build(meta):
    import concourse.bass as bass
    import concourse.bacc as bacc
    import concourse.mybir as mybir
    import concourse.tile as tile
    from concourse.masks import make_identity

    T = meta["T"]
    twin = meta["twin"]
    woff = meta["woff"]
    P = meta["P"]
    fdt = mybir.dt.float32
    bdt = mybir.dt.bfloat16
    AF = mybir.ActivationFunctionType
    OP = mybir.AluOpType

    nc = bacc.Bacc("TRN2", target_bir_lowering=False, debug=False)
    xT_d = nc.dram_tensor("xT", [D_IN, TROWS], bdt, kind="ExternalInput")
    Wcomb_d = nc.dram_tensor("Wcomb", [D_IN, 2 * D_OUT], bdt, kind="ExternalInput")
    W1eT_d = nc.dram_tensor("W1eT", [D_EDGE, D_OUT], bdt, kind="ExternalInput")
    sent_d = nc.dram_tensor("sent_row", [1, TW], bdt, kind="ExternalInput")
    iota_d = nc.dram_tensor("iota_row", [128, WIN], bdt, kind="ExternalInput")
    bias_d = nc.dram_tensor("bias_rep", [128, D_OUT], fdt, kind="ExternalInput")
    eaT_d = nc.dram_tensor("eaT", [D_EDGE, T * 128], bdt, kind="ExternalInput")
    srcp_d = nc.dram_tensor("srcpan", [128, T], mybir.dt.int32, kind="ExternalInput")
    dstw_d = nc.dram_tensor("dstwpan", [128, T], bdt, kind="ExternalInput")
    table_d = nc.dram_tensor("table", [TROWS, TW], bdt)
    out_d = nc.dram_tensor("out", [NPC, D_OUT], fdt, kind="ExternalOutput")

    ntile_tab = TROWS // 128

    with tile.TileContext(nc) as tc:
        with tc.tile_pool(name="const", bufs=1) as cpool:
            Wcomb_s = cpool.tile([D_IN, 2 * D_OUT], bdt)
            nc.sync.dma_start(out=Wcomb_s[:], in_=Wcomb_d[:])
            W1eT_s = cpool.tile([D_EDGE, D_OUT], bdt)
            nc.sync.dma_start(out=W1eT_s[:], in_=W1eT_d[:])
            iota_s = cpool.tile([128, WIN], bdt)
            nc.sync.dma_start(out=iota_s[:], in_=iota_d[:])
            bias_s = cpool.tile([128, D_OUT], fdt)
            nc.sync.dma_start(out=bias_s[:], in_=bias_d[:])
            sent_s = cpool.tile([1, TW], bdt)
            nc.sync.dma_start(out=sent_s[:], in_=sent_d[:])
            ident = cpool.tile([128, 128], fdt)
            make_identity(nc, ident[:])

            # ---- phase 0: table[r] = [x@W1x''^T | 1 | x@W2^T | pad] ----
            with (
                tc.tile_pool(name="p0s", bufs=4) as p0s,
                tc.tile_pool(name="p0p", bufs=3, space="PSUM") as p0p,
            ):
                XB = 8  # table tiles per DMA group (keeps DMA count low)
                for g in range(ntile_tab // XB):
                    r0 = g * XB * 128
                    xt_t = p0s.tile([D_IN, XB * 128], bdt, tag="xt")
                    nc.sync.dma_start(out=xt_t[:], in_=xT_d[:, r0:r0 + XB * 128])
                    ps = p0p.tile([128, XB * 2 * D_OUT], fdt, tag="ps")
                    for k in range(XB):
                        nc.tensor.matmul(
                            out=ps[:, k * 2 * D_OUT:(k + 1) * 2 * D_OUT],
                            lhsT=xt_t[:, k * 128:(k + 1) * 128],
                            rhs=Wcomb_s[:], start=True, stop=True)
                    ob = p0s.tile([128, XB * TW], bdt, tag="ob")
                    obv = ob[:].rearrange("p (k w) -> p k w", k=XB)
                    psv = ps[:].rearrange("p (k d) -> p k d", k=XB)
                    nc.vector.tensor_copy(out=obv[:, :, 0:D_OUT], in_=psv[:, :, 0:D_OUT])
                    nc.vector.memset(obv[:, :, D_OUT:D_OUT + 1], 1.0)
                    nc.vector.memset(obv[:, :, GW:TW], 0.0)
                    nc.vector.tensor_copy(out=obv[:, :, D_OUT + 1:GW],
                                          in_=psv[:, :, D_OUT:2 * D_OUT])
                    nc.sync.dma_start(
                        out=table_d[r0:r0 + XB * 128, :].rearrange(
                            "(k p) w -> p k w", p=128),
                        in_=obv)
                nc.sync.dma_start(out=table_d[SENT:SENT + 1, :], in_=sent_s[:])

            # ---- edge phase ----
            with (
                tc.tile_pool(name="gat", bufs=16) as gat,
                tc.tile_pool(name="eap", bufs=2) as eap,
                tc.tile_pool(name="pan", bufs=2) as pan,
                tc.tile_pool(name="wrk", bufs=6) as wrk,
                tc.tile_pool(name="psf", bufs=3, space="PSUM") as psf,
                tc.tile_pool(name="psh", bufs=2, space="PSUM") as psh,
                tc.tile_pool(name="pst", bufs=1, space="PSUM") as pst,
                tc.tile_pool(name="drn", bufs=2) as drn,
            ):
                ea_ch = None
                src_ch = None
                dstw_ch = None
                for w in range(NWIN):
                    tw = twin[w]
                    t0 = woff[w]
                    psH = psh.tile([D_OUT + 1, WIN], fdt, tag="psH")
                    for j in range(tw):
                        t = t0 + j
                        if t % EA_CH == 0:
                            ea_ch = eap.tile([D_EDGE, EA_CH * 128], bdt, tag="ea")
                            hi = min((t + EA_CH) * 128, T * 128)
                            nc.sync.dma_start(out=ea_ch[:, :hi - t * 128],
                                              in_=eaT_d[:, t * 128:hi])
                        if t % PAN_CH == 0:
                            hi = min(t + PAN_CH, T)
                            src_ch = pan.tile([128, PAN_CH], mybir.dt.int32, tag="srcp")
                            nc.sync.dma_start(out=src_ch[:, :hi - t], in_=srcp_d[:, t:hi])
                            dstw_ch = pan.tile([128, PAN_CH], bdt, tag="dstwp")
                            nc.sync.dma_start(out=dstw_ch[:, :hi - t], in_=dstw_d[:, t:hi])
                        co = t % EA_CH
                        po = t % PAN_CH

                        uxt = gat.tile([128, GW], bdt, tag="uxt")
                        nc.gpsimd.indirect_dma_start(
                            out=uxt[:],
                            out_offset=None,
                            in_=table_d[:],
                            in_offset=bass.IndirectOffsetOnAxis(
                                ap=src_ch[:, po:po + 1], axis=0),
                        )
                        psF = psf.tile([128, D_OUT], fdt, tag="psF")
                        nc.tensor.matmul(out=psF[:],
                                         lhsT=ea_ch[:, co * 128:(co + 1) * 128],
                                         rhs=W1eT_s[:], start=True, stop=True)
                        fpr = wrk.tile([128, D_OUT], fdt, tag="fpr")
                        nc.vector.tensor_add(out=fpr[:], in0=psF[:], in1=uxt[:, :D_OUT])
                        st = wrk.tile([128, 4], fdt, tag="st")
                        scr = wrk.tile([128, D_OUT], fdt, tag="scr")
                        # l = sum max(f'',.01f'')[:P] + sum min(f'',.01f'')[P:]
                        if P > 0:
                            nc.vector.scalar_tensor_tensor(
                                out=scr[:, :P], in0=fpr[:, :P], scalar=NEG,
                                in1=fpr[:, :P], op0=OP.mult, op1=OP.max,
                                accum_out=st[:, 0:1])
                        else:
                            nc.vector.memset(st[:, 0:1], 0.0)
                        if P < D_OUT:
                            nc.vector.scalar_tensor_tensor(
                                out=scr[:, P:D_OUT], in0=fpr[:, P:D_OUT], scalar=NEG,
                                in1=fpr[:, P:D_OUT], op0=OP.mult, op1=OP.min,
                                accum_out=st[:, 1:2])
                        else:
                            nc.vector.memset(st[:, 1:2], 0.0)
                        nc.scalar.activation(out=st[:, 2:3], in_=st[:, 1:2],
                                             func=AF.Exp, bias=st[:, 0:1])
                        pay = wrk.tile([128, D_OUT + 1], bdt, tag="pay")
                        nc.scalar.activation(out=pay[:], in_=uxt[:, D_OUT:GW],
                                             func=AF.Copy, scale=st[:, 2:3])
                        oh = wrk.tile([128, WIN], bdt, tag="oh")
                        nc.vector.tensor_tensor(
                            out=oh[:],
                            in0=dstw_ch[:, po:po + 1].to_broadcast([128, WIN]),
                            in1=iota_s[:],
                            op=OP.is_equal)
                        nc.tensor.matmul(out=psH[:], lhsT=pay[:], rhs=oh[:],
                                         start=(j == 0), stop=(j == tw - 1))
                    # ---- drain window w (two 128-node halves) ----
                    sbH = drn.tile([D_OUT + 1, WIN], fdt, tag="sbH")
                    nc.scalar.copy(out=sbH[:], in_=psH[:])
                    for hh in range(WIN // 128):
                        r0 = w * WIN + hh * 128
                        if r0 >= NPC:
                            break
                        rows = min(128, NPC - r0)
                        psHT = pst.tile([128, D_OUT + 1], fdt, tag="psHT")
                        nc.tensor.transpose(
                            out=psHT[:], in_=sbH[:, hh * 128:(hh + 1) * 128],
                            identity=ident[:D_OUT + 1, :D_OUT + 1])
                        dn = drn.tile([128, 2], fdt, tag="dn")
                        nc.vector.tensor_scalar_max(out=dn[:, 0:1], in0=psHT[:, 0:1],
                                                    scalar1=1e-30)
                        nc.vector.reciprocal(out=dn[:, 1:2], in_=dn[:, 0:1])
                        ot = drn.tile([128, D_OUT], fdt, tag="ot")
                        nc.scalar.activation(out=ot[:], in_=psHT[:, 1:D_OUT + 1],
                                             func=AF.Copy, scale=dn[:, 1:2])
                        ot2 = drn.tile([128, D_OUT], fdt, tag="ot2")
                        nc.vector.tensor_add(out=ot2[:], in0=ot[:], in1=bias_s[:])
                        nc.sync.dma_start(out=out_d[r0:r0 + rows, :], in_=ot2[:rows, :])
    nc.compile()
    return nc


def kernel(**inputs):
    in_maps, meta = _preprocess(**inputs)
    nc = _build(meta)
    from concourse import bass_utils
    res = bass_utils.run_bass_kernel_spmd(nc, in_maps, core_ids=list(range(N_CORES)))
    h = np.concatenate([res.results[c]["out"] for c in range(N_CORES)], axis=0)
    return h.astype(np.float32)
